# revision 19
# baseline (speedup 1.0000x reference)
"""DeepSeekMoE (E=8, top-2) on 8 TRN2 NeuronCores.

Host computes the gate (routing IS the data-dependent sharding step) and
gathers each expert's tokens, concatenated expert-by-expert with per-expert
capacities C_e = exact count rounded to 16. Expert FFNs run 8-way
tensor-parallel over the hidden dim: every core holds a 512-wide H-slice of
ALL experts' w1/w2 and processes every routed token exactly once, producing
partial y sums (b2/8 folded into each partial keeps the bias exact in f32).
The shared expert runs data-parallel (512 tokens per core, full H). Host
sums the 8 partial slices and scatter-adds into the output.

Per-expert capacities depend on the routing but are identical on all cores,
so one SPMD program serves all 8. Activations stay transposed
([feature, token]) so both matmuls use natural-layout weights as the
stationary operand — no on-device transposes. Matmuls are bf16 with f32
PSUM accumulation; gelu+bias fuses on ScalarE at PSUM eviction; the second
matmul's eviction fuses (y + b2/8) * combine_weight on VectorE. All loads
stream just-in-time on one queue in need-order.
"""

import numpy as np
import ml_dtypes

import concourse.mybir as mybir
import concourse.tile as tile
from concourse import bacc
from concourse.bass_utils import run_bass_kernel_spmd

D = 1024
E = 8
TOPK = 2
H = 4096
NCORES = 8
P = 128
NCHUNK = 512
HL = H // NCORES          # 512: per-core H-slice for expert TP
NHL = HL // P             # 4 local h-tiles
NKC = HL // P             # 4 local k-chunks in the second matmul

BF16 = mybir.dt.bfloat16
F32 = mybir.dt.float32

_cache: dict = {}


def _chunks(c):
    out = []
    o = 0
    while o < c:
        n = min(NCHUNK, c - o)
        out.append((o, n))
        o += n
    return out


def build(caps: tuple, S: int):
    """Build + compile the SPMD per-core program.

    caps: per-expert token capacities (multiples of 16, may be 0).
    S: shared-expert tokens per core.
    """
    nc = bacc.Bacc(None, target_bir_lowering=False, debug=False)

    ND = D // P      # 8 d-chunks
    NH = H // P      # 32 h-chunks (shared expert, full H)
    NHQ = NH // 4    # 8 h-quad groups for the shared expert
    CT = sum(caps)
    offs = [0]
    for c in caps:
        offs.append(offs[-1] + c)

    xg = nc.declare_dram_parameter("xg", [D, CT], BF16, isOutput=False)
    sx = nc.declare_dram_parameter("sx", [D, S], BF16, isOutput=False)
    # expert weights: this core's H-slice of every expert
    w1 = nc.declare_dram_parameter("w1", [E, ND, P, HL], BF16, isOutput=False)
    w2 = nc.declare_dram_parameter("w2", [E, NKC, P, D], BF16, isOutput=False)
    # shared-expert weights: full H, chunked
    sw1 = nc.declare_dram_parameter("sw1", [NHQ, ND, P, 512], BF16, isOutput=False)
    sw2 = nc.declare_dram_parameter("sw2", [ND, NHQ, P, 4, P], BF16, isOutput=False)
    b1c = nc.declare_dram_parameter("b1c", [P, E * NHL], F32, isOutput=False)
    b2c = nc.declare_dram_parameter("b2c", [P, E * ND], F32, isOutput=False)
    sb1c = nc.declare_dram_parameter("sb1c", [P, NH], F32, isOutput=False)
    sb2c = nc.declare_dram_parameter("sb2c", [P, ND], F32, isOutput=False)
    scale = nc.declare_dram_parameter("scale", [P, CT], F32, isOutput=False)
    ye = nc.declare_dram_parameter("ye", [D, CT], BF16, isOutput=True)
    ys = nc.declare_dram_parameter("ys", [D, S], F32, isOutput=True)

    sch = _chunks(S)

    with tile.TileContext(nc) as tc:
        with (
            tc.tile_pool(name="wp1", bufs=24) as wp1,
            tc.tile_pool(name="wp2", bufs=12) as wp2,
            tc.tile_pool(name="xp", bufs=24) as xp,
            tc.tile_pool(name="sp", bufs=8) as sp,
            tc.tile_pool(name="hp", bufs=8) as hp,
            tc.tile_pool(name="shp", bufs=32) as shp,
            tc.tile_pool(name="scp", bufs=8) as scp,
            tc.tile_pool(name="cp", bufs=1) as cp,
            tc.tile_pool(name="op", bufs=4) as op,
            tc.tile_pool(name="pp", bufs=8, space="PSUM") as pp,
        ):
            b1t = cp.tile([P, E * NHL], F32, tag="b1")
            sb1t = cp.tile([P, NH], F32, tag="sb1")
            b2t = cp.tile([P, E * ND], F32, tag="b2")
            sb2t = cp.tile([P, ND], F32, tag="sb2")
            sxt = [sp.tile([P, S], BF16, tag="sx", name=f"sx{d}")
                   for d in range(ND)]
            live = [ex for ex in range(E) if caps[ex] > 0]

            # ------------- expert FFNs, 8-way TP over H, per-expert -------
            for ei, ex in enumerate(live):
                cap = caps[ex]
                chs = _chunks(cap)
                # DMA issue order = need order: w1_e + xg_e gate phase A,
                # w2_e + scale_e are needed ~25us later in phase B
                w1ts = []
                xts = {}
                for d in range(ND):
                    t = wp1.tile([P, HL], BF16, tag="w1", name=f"w1_{ex}_{d}")
                    nc.sync.dma_start(t[:], w1[ex, d])
                    w1ts.append(t)
                    for ti, (o, n) in enumerate(chs):
                        xt = xp.tile([P, NCHUNK], BF16, tag="xg",
                                     name=f"xg_{ex}_{d}_{ti}")
                        nc.sync.dma_start(
                            xt[:, :n],
                            xg[d * P:(d + 1) * P, offs[ex] + o:offs[ex] + o + n])
                        xts[(d, ti)] = xt
                if ei == 0:
                    nc.sync.dma_start(b1t[:], b1c[:])
                    nc.sync.dma_start(b2t[:], b2c[:])
                w2ts = []
                for kc in range(NKC):
                    t = wp2.tile([P, D], BF16, tag="w2", name=f"w2_{ex}_{kc}")
                    nc.sync.dma_start(t[:], w2[ex, kc])
                    w2ts.append(t)
                scts = []
                for ti, (o, n) in enumerate(chs):
                    st = scp.tile([P, NCHUNK], F32, tag="sc", name=f"sc_{ex}_{ti}")
                    nc.sync.dma_start(
                        st[:, :n], scale[:, offs[ex] + o:offs[ex] + o + n])
                    scts.append(st)
                if ei == 3:
                    # prefetch shared-expert inputs behind expert streams
                    nc.sync.dma_start(sb1t[:], sb1c[:])
                    for d in range(ND):
                        nc.sync.dma_start(sxt[d][:], sx[d * P:(d + 1) * P, :])

                # phase A (chunk-outer): hT_e = gelu(w1_e.T @ xg_e + b1_e)
                hts = [hp.tile([P, cap], BF16, tag="h", name=f"h_{ex}_{h}")
                       for h in range(NHL)]
                for ti, (o, n) in enumerate(chs):
                    for h in range(NHL):
                        ps = pp.tile([P, n], F32, tag="ps", name=f"psA{h}")
                        for d in range(ND):
                            nc.tensor.matmul(
                                ps[:, :n],
                                w1ts[d][:, h * P:(h + 1) * P],
                                xts[(d, ti)][:, :n],
                                start=(d == 0),
                                stop=(d == ND - 1),
                            )
                        nc.scalar.activation(
                            hts[h][:, o:o + n],
                            ps[:, :n],
                            mybir.ActivationFunctionType.Gelu,
                            bias=b1t[:, ex * NHL + h:ex * NHL + h + 1],
                        )

                # phase B: partial y_e = (w2_e.T @ hT_e + b2_e/8) * p
                for dt in range(ND):
                    psums = [pp.tile([P, n], F32, tag="ps", name=f"psB{ti}")
                             for ti, (_, n) in enumerate(chs)]
                    for kc in range(NKC):
                        for ti, (o, n) in enumerate(chs):
                            nc.tensor.matmul(
                                psums[ti][:, :n],
                                w2ts[kc][:, dt * P:(dt + 1) * P],
                                hts[kc][:, o:o + n],
                                start=(kc == 0),
                                stop=(kc == NKC - 1),
                            )
                    for ti, (o, n) in enumerate(chs):
                        ot = op.tile([P, NCHUNK], BF16, tag="o", name=f"o{ti}")
                        nc.vector.scalar_tensor_tensor(
                            ot[:, :n],
                            psums[ti][:, :n],
                            b2t[:, ex * ND + dt:ex * ND + dt + 1],
                            scts[ti][:, :n],
                            mybir.AluOpType.add,
                            mybir.AluOpType.mult,
                        )
                        nc.sync.dma_start(
                            ye[dt * P:(dt + 1) * P,
                               offs[ex] + o:offs[ex] + o + n],
                            ot[:, :n])

            # ------------- shared expert, data-parallel, full H -----------
            shts = []
            for hq in range(NHQ):
                w1ts = []
                for d in range(ND):
                    t = wp1.tile([P, 512], BF16, tag="w1", name=f"sw1_{hq}_{d}")
                    nc.sync.dma_start(t[:], sw1[hq, d])
                    w1ts.append(t)
                for hh in range(4):
                    h = hq * 4 + hh
                    ht = shp.tile([P, S], BF16, tag="sh", name=f"sh{h}")
                    psums = [pp.tile([P, n], F32, tag="ps", name=f"psC{ti}")
                             for ti, (_, n) in enumerate(sch)]
                    for d in range(ND):
                        for ti, (o, n) in enumerate(sch):
                            nc.tensor.matmul(
                                psums[ti][:, :n],
                                w1ts[d][:, hh * P:(hh + 1) * P],
                                sxt[d][:, o:o + n],
                                start=(d == 0),
                                stop=(d == ND - 1),
                            )
                    for ti, (o, n) in enumerate(sch):
                        nc.scalar.activation(
                            ht[:, o:o + n],
                            psums[ti][:, :n],
                            mybir.ActivationFunctionType.Gelu,
                            bias=sb1t[:, h:h + 1],
                        )
                    shts.append(ht)

            nc.sync.dma_start(sb2t[:], sb2c[:])
            for dt in range(ND):
                w2ts = []
                for j in range(NHQ):
                    t = wp2.tile([P, 4, P], BF16, tag="w2", name=f"sw2_{dt}_{j}")
                    nc.sync.dma_start(t[:], sw2[dt, j])
                    w2ts.append(t)
                psums = [pp.tile([P, n], F32, tag="ps", name=f"psD{ti}")
                         for ti, (_, n) in enumerate(sch)]
                for h in range(NH):
                    j, a = divmod(h, 4)
                    for ti, (o, n) in enumerate(sch):
                        nc.tensor.matmul(
                            psums[ti][:, :n],
                            w2ts[j][:, a, :],
                            shts[h][:, o:o + n],
                            start=(h == 0),
                            stop=(h == NH - 1),
                        )
                for ti, (o, n) in enumerate(sch):
                    ot = op.tile([P, NCHUNK], F32, tag="os", name=f"os{ti}")
                    nc.vector.tensor_scalar_add(
                        ot[:, :n], psums[ti][:, :n], sb2t[:, dt:dt + 1])
                    nc.sync.dma_start(
                        ys[dt * P:(dt + 1) * P, o:o + n], ot[:, :n])

    nc.compile()
    return nc


def _get_nc(caps, S):
    key = (caps, S)
    if key not in _cache:
        _cache[key] = build(caps, S)
    return _cache[key]


def prepare(x, gate_w, gate_b, route_bias, shared_w1, shared_b1, shared_w2,
            shared_b2, exp_w1, exp_b1, exp_w2, exp_b2):
    """Host routing + sharding. Returns (nc, in_maps, combine_fn)."""
    B, SEQ, _ = x.shape
    T = B * SEQ
    S = T // NCORES
    ND = D // P
    xf = np.ascontiguousarray(x.reshape(T, D)).astype(np.float32)

    # --- gate / routing (this IS the data-dependent shard map) ---
    logits = xf @ np.asarray(gate_w, np.float32) + np.asarray(gate_b, np.float32) \
        + np.asarray(route_bias, np.float32)
    m = logits.max(axis=-1, keepdims=True)
    e = np.exp(logits - m)
    probs = e / e.sum(axis=-1, keepdims=True)
    i1 = probs.argmax(axis=-1)
    p1 = probs[np.arange(T), i1]
    probs2 = probs.copy()
    probs2[np.arange(T), i1] = -np.inf
    i2 = probs2.argmax(axis=-1)
    p2 = probs[np.arange(T), i2]
    den = p1 + p2
    p1n = p1 / den
    p2n = p2 / den

    idx = []
    pv = []
    for ex in range(E):
        sel1 = np.nonzero(i1 == ex)[0]
        sel2 = np.nonzero(i2 == ex)[0]
        idx.append(np.concatenate([sel1, sel2]))
        pv.append(np.concatenate([p1n[sel1], p2n[sel2]]).astype(np.float32))
    counts = np.array([len(ix) for ix in idx])
    caps = tuple(int(np.ceil(c / 16)) * 16 for c in counts)
    CT = sum(caps)
    offs = np.concatenate([[0], np.cumsum(caps)]).astype(int)

    xf_bf = xf.astype(ml_dtypes.bfloat16)

    # expert-concatenated gathered tokens + combine weights (same on cores)
    xg = np.zeros((D, CT), ml_dtypes.bfloat16)
    sc = np.zeros((P, CT), np.float32)
    for ex in range(E):
        n = counts[ex]
        xg[:, offs[ex]:offs[ex] + n] = xf_bf[idx[ex]].T
        sc[:, offs[ex]:offs[ex] + n] = pv[ex][None, :]

    ew1 = np.asarray(exp_w1, np.float32)   # [E, D, H]
    ew2 = np.asarray(exp_w2, np.float32)   # [E, H, D]
    eb1 = np.asarray(exp_b1, np.float32)   # [E, H]
    # b2/8 per partial; [P, E*ND] with column ex*ND+dt
    b2r = np.ascontiguousarray(
        np.asarray(exp_b2, np.float32).reshape(E, ND, P)
        .transpose(2, 0, 1).reshape(P, E * ND) / NCORES)
    sw1_p = np.ascontiguousarray(
        np.asarray(shared_w1, np.float32).reshape(D // P, P, H // 512, 512)
        .transpose(2, 0, 1, 3)).astype(ml_dtypes.bfloat16)
    sw2_p = np.ascontiguousarray(
        np.asarray(shared_w2, np.float32).reshape(H // 512, 4, P, D // P, P)
        .transpose(3, 0, 2, 1, 4)).astype(ml_dtypes.bfloat16)
    sb1c = np.ascontiguousarray(np.asarray(shared_b1, np.float32).reshape(H // P, P).T)
    sb2c = np.ascontiguousarray(np.asarray(shared_b2, np.float32).reshape(D // P, P).T)

    in_maps = []
    for r in range(NCORES):
        lo = r * HL
        in_maps.append({
            "xg": xg,
            "sx": np.ascontiguousarray(xf_bf[r * S:(r + 1) * S].T),
            "w1": np.ascontiguousarray(
                ew1[:, :, lo:lo + HL].reshape(E, ND, P, HL)
            ).astype(ml_dtypes.bfloat16),
            "w2": np.ascontiguousarray(
                ew2[:, lo:lo + HL, :].reshape(E, NKC, P, D)
            ).astype(ml_dtypes.bfloat16),
            "sw1": sw1_p,
            "sw2": sw2_p,
            "b1c": np.ascontiguousarray(
                eb1[:, lo:lo + HL].reshape(E * NHL, P).T),
            "b2c": b2r,
            "sb1c": sb1c,
            "sb2c": sb2c,
            "scale": sc,
        })

    nc = _get_nc(caps, S)

    def combine(results):
        out = np.zeros((T, D), np.float32)
        for r in range(NCORES):
            out[r * S:(r + 1) * S] = results[r]["ys"].T
        acc = results[0]["ye"].astype(np.float32)
        for r in range(1, NCORES):
            acc += results[r]["ye"].astype(np.float32)
        for ex in range(E):
            n = counts[ex]
            out[idx[ex]] += acc[:, offs[ex]:offs[ex] + n].T
        return out.reshape(B, SEQ, D)

    return nc, in_maps, combine


def kernel(**inputs):
    nc, in_maps, combine = prepare(**inputs)
    res = run_bass_kernel_spmd(nc, in_maps, core_ids=list(range(NCORES)))
    return combine(res.results)


# revision 20
# speedup vs baseline: 1.1516x; 1.1516x over previous
"""DeepSeekMoE (E=8, top-2) on 8 TRN2 NeuronCores.

Host computes the gate (routing IS the data-dependent sharding step) and
gathers each expert's tokens, concatenated expert-by-expert with per-expert
capacities C_e = exact count rounded to 16. Expert FFNs run 8-way
tensor-parallel over the hidden dim: every core holds a 512-wide H-slice of
ALL experts' w1/w2 and processes every routed token exactly once, producing
partial y sums (b2/8 folded into each partial keeps the bias exact in f32).
The shared expert runs data-parallel (512 tokens per core, full H). Host
sums the 8 partial slices and scatter-adds into the output.

Per-expert capacities depend on the routing but are identical on all cores,
so one SPMD program serves all 8. Activations stay transposed
([feature, token]) so both matmuls use natural-layout weights as the
stationary operand — no on-device transposes. Matmuls are bf16 with f32
PSUM accumulation; gelu+bias fuses on ScalarE at PSUM eviction; the second
matmul's eviction fuses (y + b2/8) * combine_weight on VectorE. All loads
stream just-in-time on one queue in need-order.
"""

import numpy as np
import ml_dtypes

import concourse.mybir as mybir
import concourse.tile as tile
from concourse import bacc
from concourse.bass_utils import run_bass_kernel_spmd

D = 1024
E = 8
TOPK = 2
H = 4096
NCORES = 8
P = 128
NCHUNK = 512
HL = H // NCORES          # 512: per-core H-slice for expert TP
NHL = HL // P             # 4 local h-tiles
NKC = HL // P             # 4 local k-chunks in the second matmul

BF16 = mybir.dt.bfloat16
F32 = mybir.dt.float32

_cache: dict = {}


def _chunks(c):
    out = []
    o = 0
    while o < c:
        n = min(NCHUNK, c - o)
        out.append((o, n))
        o += n
    return out


def build(caps: tuple, S: int):
    """Build + compile the SPMD per-core program.

    caps: per-expert token capacities (multiples of 16, may be 0).
    S: shared-expert tokens per core.
    """
    nc = bacc.Bacc(None, target_bir_lowering=False, debug=False)

    ND = D // P      # 8 d-chunks
    NH = H // P      # 32 h-chunks (shared expert, full H)
    NHQ = NH // 4    # 8 h-quad groups for the shared expert
    CT = sum(caps)
    offs = [0]
    for c in caps:
        offs.append(offs[-1] + c)

    chl = []            # global chunk table: (expert, local offset, n)
    for ex in range(E):
        for (o, n) in _chunks(caps[ex]):
            chl.append((ex, o, n))
    CHT = len(chl)
    xg = nc.declare_dram_parameter("xg", [CHT, D // P, P, NCHUNK], BF16,
                                   isOutput=False)
    sx = nc.declare_dram_parameter("sx", [D, S], BF16, isOutput=False)
    # expert weights: this core's H-slice of every expert
    w1 = nc.declare_dram_parameter("w1", [E, ND, P, HL], BF16, isOutput=False)
    w2 = nc.declare_dram_parameter("w2", [E, NKC, P, D], BF16, isOutput=False)
    # shared-expert weights: full H, chunked
    sw1 = nc.declare_dram_parameter("sw1", [NHQ, ND, P, 512], BF16, isOutput=False)
    sw2 = nc.declare_dram_parameter("sw2", [ND, NHQ, P, 4, P], BF16, isOutput=False)
    b1c = nc.declare_dram_parameter("b1c", [P, E * NHL], F32, isOutput=False)
    b2c = nc.declare_dram_parameter("b2c", [P, E * ND], F32, isOutput=False)
    sb1c = nc.declare_dram_parameter("sb1c", [P, NH], F32, isOutput=False)
    sb2c = nc.declare_dram_parameter("sb2c", [P, ND], F32, isOutput=False)
    scale = nc.declare_dram_parameter("scale", [CHT, P, NCHUNK], F32,
                                      isOutput=False)
    ye = nc.declare_dram_parameter("ye", [D // P, CHT, P, NCHUNK], BF16,
                                   isOutput=True)
    ys = nc.declare_dram_parameter("ys", [D, S], F32, isOutput=True)

    sch = _chunks(S)

    with tile.TileContext(nc) as tc:
        with (
            tc.tile_pool(name="wp1", bufs=24) as wp1,
            tc.tile_pool(name="wp2", bufs=12) as wp2,
            tc.tile_pool(name="xp", bufs=24) as xp,
            tc.tile_pool(name="sp", bufs=8) as sp,
            tc.tile_pool(name="hp", bufs=8) as hp,
            tc.tile_pool(name="shp", bufs=32) as shp,
            tc.tile_pool(name="scp", bufs=8) as scp,
            tc.tile_pool(name="cp", bufs=1) as cp,
            tc.tile_pool(name="op", bufs=4) as op,
            tc.tile_pool(name="pp", bufs=8, space="PSUM") as pp,
        ):
            b1t = cp.tile([P, E * NHL], F32, tag="b1")
            sb1t = cp.tile([P, NH], F32, tag="sb1")
            b2t = cp.tile([P, E * ND], F32, tag="b2")
            sb2t = cp.tile([P, ND], F32, tag="sb2")
            sxt = [sp.tile([P, S], BF16, tag="sx", name=f"sx{d}")
                   for d in range(ND)]
            live = [ex for ex in range(E) if caps[ex] > 0]

            # ------------- expert FFNs, 8-way TP over H, per-expert -------
            cbase = {}
            ci_run = 0
            for ex in range(E):
                nch = len(_chunks(caps[ex]))
                cbase[ex] = ci_run
                ci_run += nch
            for ei, ex in enumerate(live):
                cap = caps[ex]
                chs = _chunks(cap)
                # DMA issue order = need order: w1_e + xg_e gate phase A,
                # w2_e + scale_e are needed ~25us later in phase B
                w1ts = []
                xts = {}
                for d in range(ND):
                    t = wp1.tile([P, HL], BF16, tag="w1", name=f"w1_{ex}_{d}")
                    nc.sync.dma_start(t[:], w1[ex, d])
                    w1ts.append(t)
                    for ti, (o, n) in enumerate(chs):
                        xt = xp.tile([P, NCHUNK], BF16, tag="xg",
                                     name=f"xg_{ex}_{d}_{ti}")
                        nc.sync.dma_start(
                            xt[:, :n], xg[cbase[ex] + ti, d, :, :n])
                        xts[(d, ti)] = xt
                if ei == 0:
                    nc.sync.dma_start(b1t[:], b1c[:])
                    nc.sync.dma_start(b2t[:], b2c[:])
                w2ts = []
                for kc in range(NKC):
                    t = wp2.tile([P, D], BF16, tag="w2", name=f"w2_{ex}_{kc}")
                    nc.sync.dma_start(t[:], w2[ex, kc])
                    w2ts.append(t)
                scts = []
                for ti, (o, n) in enumerate(chs):
                    st = scp.tile([P, NCHUNK], F32, tag="sc", name=f"sc_{ex}_{ti}")
                    nc.sync.dma_start(
                        st[:, :n], scale[cbase[ex] + ti, :, :n])
                    scts.append(st)
                if ei == 3:
                    # prefetch shared-expert inputs behind expert streams
                    nc.sync.dma_start(sb1t[:], sb1c[:])
                    for d in range(ND):
                        nc.sync.dma_start(sxt[d][:], sx[d * P:(d + 1) * P, :])

                # phase A (chunk-outer): hT_e = gelu(w1_e.T @ xg_e + b1_e)
                hts = [hp.tile([P, cap], BF16, tag="h", name=f"h_{ex}_{h}")
                       for h in range(NHL)]
                for ti, (o, n) in enumerate(chs):
                    for h in range(NHL):
                        ps = pp.tile([P, n], F32, tag="ps", name=f"psA{h}")
                        for d in range(ND):
                            nc.tensor.matmul(
                                ps[:, :n],
                                w1ts[d][:, h * P:(h + 1) * P],
                                xts[(d, ti)][:, :n],
                                start=(d == 0),
                                stop=(d == ND - 1),
                            )
                        nc.scalar.activation(
                            hts[h][:, o:o + n],
                            ps[:, :n],
                            mybir.ActivationFunctionType.Gelu,
                            bias=b1t[:, ex * NHL + h:ex * NHL + h + 1],
                        )

                # phase B: partial y_e = (w2_e.T @ hT_e + b2_e/8) * p
                for dt in range(ND):
                    psums = [pp.tile([P, n], F32, tag="ps", name=f"psB{ti}")
                             for ti, (_, n) in enumerate(chs)]
                    for kc in range(NKC):
                        for ti, (o, n) in enumerate(chs):
                            nc.tensor.matmul(
                                psums[ti][:, :n],
                                w2ts[kc][:, dt * P:(dt + 1) * P],
                                hts[kc][:, o:o + n],
                                start=(kc == 0),
                                stop=(kc == NKC - 1),
                            )
                    for ti, (o, n) in enumerate(chs):
                        ot = op.tile([P, NCHUNK], BF16, tag="o", name=f"o{ti}")
                        nc.vector.scalar_tensor_tensor(
                            ot[:, :n],
                            psums[ti][:, :n],
                            b2t[:, ex * ND + dt:ex * ND + dt + 1],
                            scts[ti][:, :n],
                            mybir.AluOpType.add,
                            mybir.AluOpType.mult,
                        )
                        nc.gpsimd.dma_start(
                            ye[dt, cbase[ex] + ti, :, :n], ot[:, :n])

            # ------------- shared expert, data-parallel, full H -----------
            shts = []
            for hq in range(NHQ):
                w1ts = []
                for d in range(ND):
                    t = wp1.tile([P, 512], BF16, tag="w1", name=f"sw1_{hq}_{d}")
                    nc.sync.dma_start(t[:], sw1[hq, d])
                    w1ts.append(t)
                for hh in range(4):
                    h = hq * 4 + hh
                    ht = shp.tile([P, S], BF16, tag="sh", name=f"sh{h}")
                    psums = [pp.tile([P, n], F32, tag="ps", name=f"psC{ti}")
                             for ti, (_, n) in enumerate(sch)]
                    for d in range(ND):
                        for ti, (o, n) in enumerate(sch):
                            nc.tensor.matmul(
                                psums[ti][:, :n],
                                w1ts[d][:, hh * P:(hh + 1) * P],
                                sxt[d][:, o:o + n],
                                start=(d == 0),
                                stop=(d == ND - 1),
                            )
                    for ti, (o, n) in enumerate(sch):
                        nc.scalar.activation(
                            ht[:, o:o + n],
                            psums[ti][:, :n],
                            mybir.ActivationFunctionType.Gelu,
                            bias=sb1t[:, h:h + 1],
                        )
                    shts.append(ht)

            nc.sync.dma_start(sb2t[:], sb2c[:])
            for dt in range(ND):
                w2ts = []
                for j in range(NHQ):
                    t = wp2.tile([P, 4, P], BF16, tag="w2", name=f"sw2_{dt}_{j}")
                    nc.sync.dma_start(t[:], sw2[dt, j])
                    w2ts.append(t)
                psums = [pp.tile([P, n], F32, tag="ps", name=f"psD{ti}")
                         for ti, (_, n) in enumerate(sch)]
                for h in range(NH):
                    j, a = divmod(h, 4)
                    for ti, (o, n) in enumerate(sch):
                        nc.tensor.matmul(
                            psums[ti][:, :n],
                            w2ts[j][:, a, :],
                            shts[h][:, o:o + n],
                            start=(h == 0),
                            stop=(h == NH - 1),
                        )
                for ti, (o, n) in enumerate(sch):
                    ot = op.tile([P, NCHUNK], F32, tag="os", name=f"os{ti}")
                    nc.vector.tensor_scalar_add(
                        ot[:, :n], psums[ti][:, :n], sb2t[:, dt:dt + 1])
                    nc.gpsimd.dma_start(
                        ys[dt * P:(dt + 1) * P, o:o + n], ot[:, :n])

    nc.compile()
    return nc


def _get_nc(caps, S):
    key = (caps, S)
    if key not in _cache:
        _cache[key] = build(caps, S)
    return _cache[key]


def prepare(x, gate_w, gate_b, route_bias, shared_w1, shared_b1, shared_w2,
            shared_b2, exp_w1, exp_b1, exp_w2, exp_b2):
    """Host routing + sharding. Returns (nc, in_maps, combine_fn)."""
    B, SEQ, _ = x.shape
    T = B * SEQ
    S = T // NCORES
    ND = D // P
    xf = np.ascontiguousarray(x.reshape(T, D)).astype(np.float32)

    # --- gate / routing (this IS the data-dependent shard map) ---
    logits = xf @ np.asarray(gate_w, np.float32) + np.asarray(gate_b, np.float32) \
        + np.asarray(route_bias, np.float32)
    m = logits.max(axis=-1, keepdims=True)
    e = np.exp(logits - m)
    probs = e / e.sum(axis=-1, keepdims=True)
    i1 = probs.argmax(axis=-1)
    p1 = probs[np.arange(T), i1]
    probs2 = probs.copy()
    probs2[np.arange(T), i1] = -np.inf
    i2 = probs2.argmax(axis=-1)
    p2 = probs[np.arange(T), i2]
    den = p1 + p2
    p1n = p1 / den
    p2n = p2 / den

    idx = []
    pv = []
    for ex in range(E):
        sel1 = np.nonzero(i1 == ex)[0]
        sel2 = np.nonzero(i2 == ex)[0]
        idx.append(np.concatenate([sel1, sel2]))
        pv.append(np.concatenate([p1n[sel1], p2n[sel2]]).astype(np.float32))
    counts = np.array([len(ix) for ix in idx])
    caps = tuple(int(np.ceil(c / 16)) * 16 for c in counts)
    CT = sum(caps)
    offs = np.concatenate([[0], np.cumsum(caps)]).astype(int)

    xf_bf = xf.astype(ml_dtypes.bfloat16)

    # expert-concatenated gathered tokens + combine weights, packed so
    # every device DMA is a dense per-chunk block (same on all cores)
    chl = []
    for ex in range(E):
        for (o, n) in _chunks(caps[ex]):
            chl.append((ex, o, n))
    CHT = len(chl)
    xg = np.zeros((CHT, D // P, P, NCHUNK), ml_dtypes.bfloat16)
    sc = np.zeros((CHT, P, NCHUNK), np.float32)
    for ci, (ex, o, n) in enumerate(chl):
        take = idx[ex][o:o + n]
        nn = len(take)
        if nn:
            xg[ci, :, :, :nn] = xf_bf[take].T.reshape(D // P, P, nn)
            sc[ci, :, :nn] = pv[ex][None, o:o + nn]

    ew1 = np.asarray(exp_w1, np.float32)   # [E, D, H]
    ew2 = np.asarray(exp_w2, np.float32)   # [E, H, D]
    eb1 = np.asarray(exp_b1, np.float32)   # [E, H]
    # b2/8 per partial; [P, E*ND] with column ex*ND+dt
    b2r = np.ascontiguousarray(
        np.asarray(exp_b2, np.float32).reshape(E, ND, P)
        .transpose(2, 0, 1).reshape(P, E * ND) / NCORES)
    sw1_p = np.ascontiguousarray(
        np.asarray(shared_w1, np.float32).reshape(D // P, P, H // 512, 512)
        .transpose(2, 0, 1, 3)).astype(ml_dtypes.bfloat16)
    sw2_p = np.ascontiguousarray(
        np.asarray(shared_w2, np.float32).reshape(H // 512, 4, P, D // P, P)
        .transpose(3, 0, 2, 1, 4)).astype(ml_dtypes.bfloat16)
    sb1c = np.ascontiguousarray(np.asarray(shared_b1, np.float32).reshape(H // P, P).T)
    sb2c = np.ascontiguousarray(np.asarray(shared_b2, np.float32).reshape(D // P, P).T)

    in_maps = []
    for r in range(NCORES):
        lo = r * HL
        in_maps.append({
            "xg": xg,
            "sx": np.ascontiguousarray(xf_bf[r * S:(r + 1) * S].T),
            "w1": np.ascontiguousarray(
                ew1[:, :, lo:lo + HL].reshape(E, ND, P, HL)
            ).astype(ml_dtypes.bfloat16),
            "w2": np.ascontiguousarray(
                ew2[:, lo:lo + HL, :].reshape(E, NKC, P, D)
            ).astype(ml_dtypes.bfloat16),
            "sw1": sw1_p,
            "sw2": sw2_p,
            "b1c": np.ascontiguousarray(
                eb1[:, lo:lo + HL].reshape(E * NHL, P).T),
            "b2c": b2r,
            "sb1c": sb1c,
            "sb2c": sb2c,
            "scale": sc,
        })

    nc = _get_nc(caps, S)

    def combine(results):
        out = np.zeros((T, D), np.float32)
        for r in range(NCORES):
            out[r * S:(r + 1) * S] = results[r]["ys"].T
        acc = results[0]["ye"].astype(np.float32)
        for r in range(1, NCORES):
            acc += results[r]["ye"].astype(np.float32)
        for ci, (ex, o, n) in enumerate(chl):
            take = idx[ex][o:o + n]
            nn = len(take)
            if nn:
                out[take] += acc[:, ci, :, :nn].reshape(D, nn).T
        return out.reshape(B, SEQ, D)

    return nc, in_maps, combine


def kernel(**inputs):
    nc, in_maps, combine = prepare(**inputs)
    res = run_bass_kernel_spmd(nc, in_maps, core_ids=list(range(NCORES)))
    return combine(res.results)


# revision 21
# speedup vs baseline: 1.2052x; 1.0465x over previous
"""DeepSeekMoE (E=8, top-2) on 8 TRN2 NeuronCores, expert-parallel.

Strategy (per sharding hint): host computes the gate (routing IS the
data-dependent sharding step), dispatches each token's top-2 experts to the
owning cores, pads per-expert token lists to a common capacity C. Core i runs
expert i's FFN over its gathered tokens plus the shared-expert FFN over a
512-token data-parallel slice. Host scatter-adds the weighted expert outputs
and shared outputs back to the full [B, S, D] tensor.

Device layout: activations live transposed ([feature, token]) end to end so
both matmuls use natural-layout weights as the stationary operand and no
on-device transposes are needed. All matmuls run in bf16 with f32 PSUM
accumulation; gelu+bias fuses on ScalarE at PSUM eviction; the second
matmul's eviction fuses (y + b2) * combine_weight on VectorE. Weights are
streamed just-in-time in small chunks (w1 by 512-wide h-column groups, w2 by
128-wide output-d groups) so the PE never waits long on weight DMA.
"""

import numpy as np
import ml_dtypes

import concourse.mybir as mybir
import concourse.tile as tile
from concourse import bacc
from concourse.bass_utils import run_bass_kernel_spmd

D = 1024
E = 8
TOPK = 2
H = 4096
NCORES = 8
P = 128
NCHUNK = 512  # moving-operand / PSUM-bank token chunk

BF16 = mybir.dt.bfloat16
F32 = mybir.dt.float32

_cache: dict = {}


def _chunks(c):
    out = []
    o = 0
    while o < c:
        n = min(NCHUNK, c - o)
        out.append((o, n))
        o += n
    return out


def build(C: int, S: int):
    """Build + compile the SPMD per-core program.

    C: expert token capacity (any multiple of 16). S: shared-expert tokens
    per core. Same program on all 8 cores; per-core data differs.
    """
    nc = bacc.Bacc(None, target_bir_lowering=False, debug=False)

    ND = D // P      # 8 d-chunks
    NH = H // P      # 32 h-chunks
    NHQ = NH // 4    # 8 h-quad groups (512 cols of w1 per group)

    xg = nc.declare_dram_parameter("xg", [D, C], BF16, isOutput=False)
    sx = nc.declare_dram_parameter("sx", [D, S], BF16, isOutput=False)
    # w1 chunked [hq, d, 128, 512]; w2 chunked [dt, j, 128, 4, 128]
    w1 = nc.declare_dram_parameter("w1", [NHQ, ND, P, 512], BF16, isOutput=False)
    w2 = nc.declare_dram_parameter("w2", [ND, NHQ, P, 4, P], BF16, isOutput=False)
    sw1 = nc.declare_dram_parameter("sw1", [NHQ, ND, P, 512], BF16, isOutput=False)
    sw2 = nc.declare_dram_parameter("sw2", [ND, NHQ, P, 4, P], BF16, isOutput=False)
    b1c = nc.declare_dram_parameter("b1c", [P, NH], F32, isOutput=False)
    b2c = nc.declare_dram_parameter("b2c", [P, ND], F32, isOutput=False)
    sb1c = nc.declare_dram_parameter("sb1c", [P, NH], F32, isOutput=False)
    sb2c = nc.declare_dram_parameter("sb2c", [P, ND], F32, isOutput=False)
    scale = nc.declare_dram_parameter("scale", [P, C], F32, isOutput=False)
    ye = nc.declare_dram_parameter("ye", [D, C], F32, isOutput=True)
    ys = nc.declare_dram_parameter("ys", [D, S], F32, isOutput=True)

    ech = _chunks(C)
    sch = _chunks(S)

    with tile.TileContext(nc) as tc:
        with (
            tc.tile_pool(name="wp1", bufs=24) as wp1,
            tc.tile_pool(name="wp2", bufs=24) as wp2,
            tc.tile_pool(name="xp", bufs=8) as xp,
            tc.tile_pool(name="sp", bufs=8) as sp,
            tc.tile_pool(name="hp", bufs=32) as hp,
            tc.tile_pool(name="cp", bufs=1) as cp,
            tc.tile_pool(name="op", bufs=4) as op,
            tc.tile_pool(name="pp", bufs=8, space="PSUM") as pp,
        ):
            # Load order = need order, all on the sync queue so descriptor
            # order delays non-critical bytes: xg + first w1 chunk gate the
            # first matmul (their DMAs interleave inside ffn); everything
            # else is issued just before first use.
            xgt = [xp.tile([P, C], BF16, tag="xg", name=f"xg{d}")
                   for d in range(ND)]
            b1t = cp.tile([P, NH], F32, tag="b1")
            sb1t = cp.tile([P, NH], F32, tag="sb1")
            b2t = cp.tile([P, ND], F32, tag="b2")
            sb2t = cp.tile([P, ND], F32, tag="sb2")
            sct = cp.tile([P, C], F32, tag="scale")
            sxt = [sp.tile([P, S], BF16, tag="sx", name=f"sx{d}")
                   for d in range(ND)]

            def ffn(tagp, w1_ap, w2_ap, b1_tile, b2_tile, x_tiles, chs, y_ap,
                    sc_tile, pre2=None, x_dma=None):
                # phase 1: hT[h, tok] = gelu(w1[:,h].T @ x + b1[h])
                hts = []
                for hq in range(NHQ):
                    w1ts = []
                    for d in range(ND):
                        t = wp1.tile([P, 512], BF16, tag="w1",
                                     name=f"{tagp}w1_{hq}_{d}")
                        nc.sync.dma_start(t[:], w1_ap[hq, d])
                        w1ts.append(t)
                        if hq == 0 and x_dma is not None:
                            x_dma(d)
                    if hq == 0:
                        nc.sync.dma_start(b1_tile[:], b1_ap_of[tagp][:])
                    for hh in range(4):
                        h = hq * 4 + hh
                        ht = hp.tile([P, C], BF16, tag="h", name=f"{tagp}h{h}")
                        psums = [pp.tile([P, n], F32, tag="ps", name=f"psA{ti}")
                                 for ti, (_, n) in enumerate(chs)]
                        for d in range(ND):
                            for ti, (o, n) in enumerate(chs):
                                nc.tensor.matmul(
                                    psums[ti][:, :n],
                                    w1ts[d][:, hh * P:(hh + 1) * P],
                                    x_tiles[d][:, o:o + n],
                                    start=(d == 0),
                                    stop=(d == ND - 1),
                                )
                        for ti, (o, n) in enumerate(chs):
                            nc.scalar.activation(
                                ht[:, o:o + n],
                                psums[ti][:, :n],
                                mybir.ActivationFunctionType.Gelu,
                                bias=b1_tile[:, h:h + 1],
                            )
                        hts.append(ht)

                # phase 2: yT[dt, tok] = (w2[:,dt].T @ hT + b2[dt]) * scale
                if pre2 is not None:
                    pre2()
                for dt in range(ND):
                    w2ts = []
                    for j in range(NHQ):
                        t = wp2.tile([P, 4, P], BF16, tag="w2",
                                     name=f"{tagp}w2_{dt}_{j}")
                        nc.sync.dma_start(t[:], w2_ap[dt, j])
                        w2ts.append(t)
                    psums = [pp.tile([P, n], F32, tag="ps", name=f"psB{ti}")
                             for ti, (_, n) in enumerate(chs)]
                    for h in range(NH):
                        j, a = divmod(h, 4)
                        for ti, (o, n) in enumerate(chs):
                            nc.tensor.matmul(
                                psums[ti][:, :n],
                                w2ts[j][:, a, :],
                                hts[h][:, o:o + n],
                                start=(h == 0),
                                stop=(h == NH - 1),
                            )
                    for ti, (o, n) in enumerate(chs):
                        pieces = [(0, n)]
                        for po, pn in pieces:
                            ot = op.tile([P, NCHUNK], F32, tag="o", name=f"o{ti}")
                            if sc_tile is not None:
                                nc.vector.scalar_tensor_tensor(
                                    ot[:, :pn],
                                    psums[ti][:, po:po + pn],
                                    b2_tile[:, dt:dt + 1],
                                    sc_tile[:, o + po:o + po + pn],
                                    mybir.AluOpType.add,
                                    mybir.AluOpType.mult,
                                )
                            else:
                                nc.vector.tensor_scalar_add(
                                    ot[:, :pn], psums[ti][:, po:po + pn],
                                    b2_tile[:, dt:dt + 1]
                                )
                            nc.sync.dma_start(
                                y_ap[dt * P:(dt + 1) * P, o + po:o + po + pn],
                                ot[:, :pn])

            b1_ap_of = {"e": b1c, "s": sb1c}

            def pre_expert_phase2():
                # loads needed by the expert epilogue and the upcoming
                # shared phases; issued here so they trail the phase-1 w1
                # stream on the queue instead of competing at t=0
                nc.sync.dma_start(b2t[:], b2c[:])
                nc.sync.dma_start(sct[:], scale[:])
                for d in range(ND):
                    nc.sync.dma_start(sxt[d][:], sx[d * P:(d + 1) * P, :])

            def pre_shared_phase2():
                nc.sync.dma_start(sb2t[:], sb2c[:])

            ffn("e", w1, w2, b1t, b2t, xgt, ech, ye, sct,
                pre2=pre_expert_phase2,
                x_dma=lambda d: nc.sync.dma_start(
                    xgt[d][:], xg[d * P:(d + 1) * P, :]))
            ffn("s", sw1, sw2, sb1t, sb2t, sxt, sch, ys, None,
                pre2=pre_shared_phase2)

    nc.compile()
    return nc


def _get_nc(C, S):
    key = (C, S)
    if key not in _cache:
        _cache[key] = build(C, S)
    return _cache[key]


def _pack_w1(w):
    # [D, H] -> [hq, d, 128, 512]
    return np.ascontiguousarray(
        np.asarray(w).reshape(D // P, P, H // 512, 512).transpose(2, 0, 1, 3)
    ).astype(ml_dtypes.bfloat16)


def _pack_w2(w):
    # [H, D] -> [dt, j, 128, 4, 128]
    return np.ascontiguousarray(
        np.asarray(w).reshape(H // 512, 4, P, D // P, P).transpose(3, 0, 2, 1, 4)
    ).astype(ml_dtypes.bfloat16)


def prepare(x, gate_w, gate_b, route_bias, shared_w1, shared_b1, shared_w2,
            shared_b2, exp_w1, exp_b1, exp_w2, exp_b2):
    """Host routing + sharding. Returns (nc, in_maps, combine_fn)."""
    B, SEQ, _ = x.shape
    T = B * SEQ
    S = T // NCORES
    xf = np.ascontiguousarray(x.reshape(T, D)).astype(np.float32)

    # --- gate / routing (this IS the data-dependent shard map) ---
    logits = xf @ np.asarray(gate_w, np.float32) + np.asarray(gate_b, np.float32) \
        + np.asarray(route_bias, np.float32)
    m = logits.max(axis=-1, keepdims=True)
    e = np.exp(logits - m)
    probs = e / e.sum(axis=-1, keepdims=True)
    i1 = probs.argmax(axis=-1)
    p1 = probs[np.arange(T), i1]
    probs2 = probs.copy()
    probs2[np.arange(T), i1] = -np.inf
    i2 = probs2.argmax(axis=-1)
    p2 = probs[np.arange(T), i2]
    den = p1 + p2
    p1n = p1 / den
    p2n = p2 / den

    idx = []
    pv = []
    for ex in range(E):
        sel1 = np.nonzero(i1 == ex)[0]
        sel2 = np.nonzero(i2 == ex)[0]
        idx.append(np.concatenate([sel1, sel2]))
        pv.append(np.concatenate([p1n[sel1], p2n[sel2]]).astype(np.float32))
    counts = np.array([len(ix) for ix in idx])
    # exact capacity rounded to 16 tokens — matmul moving dim and DMA handle
    # arbitrary sizes; only the token-chunking below cares
    C = max(16, int(np.ceil(counts.max() / 16)) * 16)

    xf_bf = xf.astype(ml_dtypes.bfloat16)
    sw1_p = _pack_w1(shared_w1)
    sw2_p = _pack_w2(shared_w2)
    sb1c = np.ascontiguousarray(np.asarray(shared_b1, np.float32).reshape(H // P, P).T)
    sb2c = np.ascontiguousarray(np.asarray(shared_b2, np.float32).reshape(D // P, P).T)

    in_maps = []
    for c in range(NCORES):
        n = counts[c]
        xg = np.zeros((D, C), ml_dtypes.bfloat16)
        xg[:, :n] = xf_bf[idx[c]].T
        sc = np.zeros((P, C), np.float32)
        sc[:, :n] = pv[c][None, :]
        in_maps.append({
            "xg": xg,
            "sx": np.ascontiguousarray(xf_bf[c * S:(c + 1) * S].T),
            "w1": _pack_w1(exp_w1[c]),
            "w2": _pack_w2(exp_w2[c]),
            "sw1": sw1_p,
            "sw2": sw2_p,
            "b1c": np.ascontiguousarray(
                np.asarray(exp_b1[c], np.float32).reshape(H // P, P).T),
            "b2c": np.ascontiguousarray(
                np.asarray(exp_b2[c], np.float32).reshape(D // P, P).T),
            "sb1c": sb1c,
            "sb2c": sb2c,
            "scale": sc,
        })

    nc = _get_nc(C, S)

    def combine(results):
        out = np.zeros((T, D), np.float32)
        for c in range(NCORES):
            out[c * S:(c + 1) * S] = results[c]["ys"].T
        for ex in range(E):
            n = counts[ex]
            out[idx[ex]] += results[ex]["ye"][:, :n].T
        return out.reshape(B, SEQ, D)

    return nc, in_maps, combine


def kernel(**inputs):
    nc, in_maps, combine = prepare(**inputs)
    res = run_bass_kernel_spmd(nc, in_maps, core_ids=list(range(NCORES)))
    return combine(res.results)
